# revision 32
# baseline (speedup 1.0000x reference)
"""Trainium2 Bass kernel for the visual-attention module.

reference:
    cv = einsum('brh,kh->brk', V, Wv)              # [B, R, R]
    cg = einsum('bth,kh->btk', h_t, Wg)            # [B, T, R]
    content = cv[:,None,:,:] + cg[:,:,None,:]      # [B, T, R, R]
    z = einsum('btrk,k->btr', tanh(content), Wh[0])
    alpha = softmax(z, -1)                         # [B, T, R]
    c_t = einsum('btr,brh->bth', alpha, V)         # [B, T, H]
    returns (c_t, alpha)

Sharding: data-parallel over batch B=64 across 8 cores (8 batches/core),
weights replicated.

Per-core dataflow (per batch b):
  - h_t/V loaded natural, PE-transposed to put H on partitions
  - cg.T computed "stacked": rows (g*64+k) for t-half g -> [113, 128] PSUM
  - content cube layout [113 partitions=(g,k), (r, t')] built by two
    broadcast-AP matmuls (identity-stack weights); tanh fused on ScalarE
    evacuating PSUM->SBUF
  - z-reduce: 13 matmuls with block-structured Wh weights accumulating into
    one [26, 512] PSUM bank (row pair 2j,2j+1 = r-chunk j)
  - z PE-transposed back to [t', (g, r)] layout, softmax there
  - alpha PE-transposed to [r, t] for the c_t matmuls against natural V
"""

import os
import sys

for _p in ("/opt/trn_rl_repo",):
    if _p not in sys.path:
        sys.path.insert(0, _p)

from contextlib import ExitStack

import numpy as np

import concourse.bass as bass
import concourse.tile as tile
from concourse import mybir
from concourse.bass_utils import run_bass_kernel_spmd

# problem sizes (hardcoded per spec)
B, T, R, H = 64, 256, 49, 1024
NCORES = 8
BPC = B // NCORES  # batches per core
G = 2  # t-halves (T = G*128)
TP = 128  # t' per half
KP = 64  # partition stride between the two g-blocks of k-rows
NP = KP + R  # 113 used partitions in the stacked layout
HC = H // 128  # h chunks
RC = 4  # r's per z-chunk (N = RC*TP = 512)
NZ = (R + RC - 1) // RC  # 13 z-chunks; last has 1 r
NDVE = 5  # trailing content groups computed on DVE instead of PE
F32 = mybir.dt.float32
BF16 = mybir.dt.bfloat16

_CACHE = {}


def _build_program():
    nc = bass.Bass()

    v_in = nc.dram_tensor("v", [BPC, R, H], F32, kind="ExternalInput")
    h_in = nc.dram_tensor("h", [BPC, T, H], F32, kind="ExternalInput")
    wv_in = nc.dram_tensor("wv", [R, H], F32, kind="ExternalInput")
    wg_in = nc.dram_tensor("wg", [R, H], F32, kind="ExternalInput")
    wh_in = nc.dram_tensor("wh", [1, R], F32, kind="ExternalInput")
    ct_out = nc.dram_tensor("ct", [BPC, T, H], F32, kind="ExternalOutput")
    al_out = nc.dram_tensor("al", [BPC, T, R], F32, kind="ExternalOutput")
    # DRAM staging for the bf16 copy of h_t; a single DRAM-source xbar
    # transpose per batch replaces 16 SBUF->SBUF 128x128 transposes
    hbf_d = nc.dram_tensor("hbf_scratch", [BPC, T, H], BF16)

    # inline constants
    ident_np = np.eye(128, dtype=np.float32)
    # lhsT_cg [113, 113]: identity on rows 0:49 and 64:113, zero elsewhere
    lcg_np = np.zeros((NP, NP), dtype=np.float32)
    for k in range(R):
        lcg_np[k, k] = 1.0
        lcg_np[KP + k, KP + k] = 1.0
    import ml_dtypes

    ident_d = nc.inline_tensor(ident_np, name="ident")
    lcg_d = nc.inline_tensor(lcg_np.astype(ml_dtypes.bfloat16), name="lcg")

    with tile.TileContext(nc) as tc, ExitStack() as ctx:
        const = ctx.enter_context(tc.tile_pool(name="const", bufs=1))
        wpool = ctx.enter_context(tc.tile_pool(name="wpool", bufs=1))
        inp = ctx.enter_context(tc.tile_pool(name="inp", bufs=3))
        trs = ctx.enter_context(tc.tile_pool(name="trs", bufs=3))
        cub = ctx.enter_context(tc.tile_pool(name="cub", bufs=2))
        sml = ctx.enter_context(tc.tile_pool(name="sml", bufs=2))
        outp = ctx.enter_context(tc.tile_pool(name="outp", bufs=2))
        ps_tr = ctx.enter_context(tc.tile_pool(name="ps_tr", bufs=2, space="PSUM"))
        ps_acc = ctx.enter_context(tc.tile_pool(name="ps_acc", bufs=2, space="PSUM"))
        ps_ct = ctx.enter_context(tc.tile_pool(name="ps_ct", bufs=2, space="PSUM"))
        ps_co = ctx.enter_context(tc.tile_pool(name="ps_co", bufs=2, space="PSUM"))

        ident = const.tile([128, 128], F32)
        nc.sync.dma_start(ident, ident_d.ap())
        lcg = const.tile([NP, NP], BF16)
        nc.sync.dma_start(lcg, lcg_d.ap())

        def tpose(out, in_, idap):
            return nc.tensor.transpose(out, in_, idap)

        def evac(dst, src):
            nc.vector.tensor_copy(dst, src)

        # --- transposed weights ---
        # wgtb [128, HC, 49] bf16 (cg matmuls run in bf16)
        # wvt2 [128, HC, 113] fp32 with Wv.T duplicated at cols 0:49 and
        # 64:113 so the cv matmul directly emits the stacked [113, 49] cv.T
        wgt = wpool.tile([128, HC, R], F32)
        wgtb = wpool.tile([128, HC, R], BF16)
        wvt2 = wpool.tile([128, HC, NP], F32)
        nc.vector.memset(wvt2, 0.0)
        for name, src, dsts in (("wg", wg_in, None), ("wv", wv_in, None)):
            wnat = inp.tile([R, H], F32, tag="wnat")
            nc.sync.dma_start(wnat, src.ap())
            for grp in range(2):
                pst = ps_tr.tile([128, 4, R], F32, tag="tr")
                for i in range(4):
                    hc = grp * 4 + i
                    tpose(
                        pst[:, i, :],
                        wnat[:, hc * 128 : (hc + 1) * 128],
                        ident[0:R, 0:R],
                    )
                sl = slice(grp * 4, (grp + 1) * 4)
                if name == "wg":
                    evac(wgt[:, sl, :], pst)
                else:
                    evac(wvt2[:, sl, 0:R], pst)
                    evac(wvt2[:, sl, KP:NP], pst)
        nc.vector.tensor_copy(wgtb, wgt)

        # --- Wh stacked column + z-reduce weights whT [113, NZ, 26] ---
        whs = wpool.tile([NP, 1], F32)
        nc.sync.dma_start(whs[0:R, :], wh_in.ap().rearrange("a b -> b a"))
        nc.sync.dma_start(whs[KP:NP, :], wh_in.ap().rearrange("a b -> b a"))
        wht = wpool.tile([NP, NZ, 2 * NZ], BF16)
        nc.vector.memset(wht, 0.0)
        for j in range(NZ):
            nc.vector.tensor_copy(wht[0:R, j, 2 * j : 2 * j + 1], whs[0:R, :])
            nc.vector.tensor_copy(wht[KP:NP, j, 2 * j + 1 : 2 * j + 2], whs[KP:NP, :])

        for b in range(BPC):
            # ---- load h_t natural, cast to bf16, DMA-xbar transpose to
            # htTb [128(h'), HC, 256(t)] bf16 ----
            htTb = trs.tile([128, HC, T], BF16, tag="htT")
            htn = inp.tile([128, G, H], F32, tag="htn")
            nc.sync.dma_start(
                htn, h_in.ap()[b].rearrange("(g t) h -> t g h", g=G)
            )
            hbf = inp.tile([128, G, H], BF16, tag="hbf")
            nc.gpsimd.tensor_copy(hbf, htn)
            nc.sync.dma_start(
                hbf_d.ap()[b].rearrange("(g t) h -> t g h", g=G), hbf
            )
            # one DRAM-source xbar transpose (kept alone on the ACT HWDGE
            # ring: mixing transpose/copy modes on a ring forces the
            # DMATranspose<->DMACopy serialization workaround):
            # out[p, hc, t] = h_t[t, hc*128+p]
            nc.scalar.dma_start_transpose(htTb, hbf_d.ap()[b])
            # ---- load V natural + transpose to vT [128, HC, 49] ----
            vnat = inp.tile([R, H], F32, tag="vnat")
            nc.sync.dma_start(vnat, v_in.ap()[b])
            vnatb = inp.tile([R, H], BF16, tag="vnatb")
            nc.gpsimd.tensor_copy(vnatb, vnat)
            vT = trs.tile([128, HC, R], F32, tag="vT")
            for grp in range(2):
                pst = ps_tr.tile([128, 4, R], F32, tag="tr")
                for i in range(4):
                    hc = grp * 4 + i
                    tpose(
                        pst[:, i, :],
                        vnat[:, hc * 128 : (hc + 1) * 128],
                        ident[0:R, 0:R],
                    )
                evac(vT[:, grp * 4 : (grp + 1) * 4, :], pst)

            # ---- cgT stacked [113, 128]: rows g*64+k = cg[b, g*128+t', k] ----
            pcg = ps_acc.tile([NP, TP], F32, tag="acc")
            for g in range(G):
                rows = pcg[0:R, :] if g == 0 else pcg[KP:NP, :]
                for hc in range(HC):
                    nc.tensor.matmul(
                        rows,
                        wgtb[:, hc, :],
                        htTb[:, hc, g * TP : (g + 1) * TP],
                        start=(hc == 0),
                        stop=(hc == HC - 1),
                    )
            # rows 49:64 of pcg are PSUM garbage; the DVE content chunks
            # read all 113 partitions, so zero the tile first and evacuate
            # the two valid row blocks (DVE partition base must be 32-aligned)
            cgs = sml.tile([NP, TP], BF16, tag="cgs")
            nc.gpsimd.memset(cgs, 0.0)
            evac(cgs[0:R, :], pcg[0:R, :])
            evac(cgs[KP:NP, :], pcg[KP:NP, :])

            # ---- cvT [49, 49] = cv[b].T ----
            pcv = ps_acc.tile([NP, R], F32, tag="acc")
            for hc in range(HC):
                nc.tensor.matmul(
                    pcv,
                    wvt2[:, hc, :],
                    vT[:, hc, 0:R],
                    start=(hc == 0),
                    stop=(hc == HC - 1),
                )
            cvt = sml.tile([NP, R], BF16, tag="cvt")
            evac(cvt, pcv)

            # ---- content cube + tanh: cube [113, 49, 128] ----
            cube = cub.tile([NP, R, TP], BF16, tag="cube")
            # PE: 4 groups of 4 r's (1 PSUM bank each); DVE: 2 groups of 16
            # plus the last r
            r_groups = [(0, 4, "pe"), (4, 4, "pe"), (8, 4, "pe"),
                        (12, 4, "pe"), (16, 33, "dve")]
            for gi, (r0, rg, where) in enumerate(r_groups):
                if where == "pe":
                    pct = ps_ct.tile([NP, RC, TP], F32, tag="pct")
                    nsub = (rg + RC - 1) // RC
                    for i in range(nsub):
                        rr0 = i * RC
                        rc = min(RC, rg - rr0)
                        rhs_cv = (
                            cvt[:, r0 + rr0 : r0 + rr0 + rc]
                            .rearrange("k (r u) -> k r u", u=1)
                            .to_broadcast([NP, rc, TP])
                        )
                        nc.tensor.matmul(
                            pct[:, rr0 : rr0 + rc, :],
                            lcg,
                            rhs_cv,
                            start=True,
                            stop=False,
                        )
                    for i in range(nsub):
                        rr0 = i * RC
                        rc = min(RC, rg - rr0)
                        rhs_cg = cgs.rearrange(
                            "p (u t) -> p u t", u=1
                        ).to_broadcast([NP, rc, TP])
                        nc.tensor.matmul(
                            pct[:, rr0 : rr0 + rc, :],
                            lcg,
                            rhs_cg,
                            start=False,
                            stop=True,
                        )
                    nc.scalar.activation(
                        out=cube[:, r0 : r0 + rg, :],
                        in_=pct[:, 0:rg, :],
                        func=mybir.ActivationFunctionType.Tanh,
                    )
                else:
                    cdv = cub.tile([NP, 33, TP], BF16, tag="cdve")
                    in_cv = (
                        cvt[:, r0 : r0 + rg]
                        .rearrange("k (r u) -> k r u", u=1)
                        .to_broadcast([NP, rg, TP])
                    )
                    in_cg = cgs.rearrange("p (u t) -> p u t", u=1).to_broadcast(
                        [NP, rg, TP]
                    )
                    nc.vector.tensor_add(cdv[:, 0:rg, :], in_cv, in_cg)
                    nc.scalar.activation(
                        out=cube[:, r0 : r0 + rg, :],
                        in_=cdv[:, 0:rg, :],
                        func=mybir.ActivationFunctionType.Tanh,
                    )

            # ---- z-reduce: psz [26, 512], row 2j+g col (rr*128+t') ----
            psz = ps_co.tile([2 * NZ, RC * TP], F32, tag="pco")
            for j in range(NZ):
                rc = min(RC, R - j * RC)
                nc.tensor.matmul(
                    psz[:, 0 : rc * TP],
                    wht[:, j, :],
                    cube[:, j * RC : j * RC + rc, :],
                    start=(j == 0),
                    stop=(j == NZ - 1),
                )
            zs = sml.tile([2 * NZ, RC * TP], F32, tag="zs")
            evac(zs, psz)

            # ---- z back to t-partition layout: z_tp [128, 2, 52] ----
            z_tp = sml.tile([TP, G, 52], F32, tag="ztp")
            # 4 PE transposes into one PSUM tile, one strided evac; the
            # out-of-range (j=12, rr>0) lanes land in z_tp's pad columns 49:52
            pzt4 = ps_tr.tile([128, RC, 2 * NZ], F32, tag="tr")
            for rr in range(RC):
                tpose(
                    pzt4[:, rr, :],
                    zs[:, rr * TP : (rr + 1) * TP],
                    ident[0 : 2 * NZ, 0 : 2 * NZ],
                )
            z4 = z_tp.rearrange("p g (j u) -> p g j u", u=RC)
            evac(
                z4.rearrange("p g j u -> p u j g"),
                pzt4.rearrange("p u (j g) -> p u j g", g=G),
            )

            # ---- softmax over r (free dim), skip max-subtraction ----
            e_tp = sml.tile([TP, G, R], F32, tag="etp")
            nc.scalar.activation(
                out=e_tp,
                in_=z_tp[:, :, 0:R],
                func=mybir.ActivationFunctionType.Exp,
            )
            ssum = sml.tile([TP, G], F32, tag="ssum")
            nc.vector.tensor_reduce(
                out=ssum, in_=e_tp, axis=mybir.AxisListType.X, op=mybir.AluOpType.add
            )
            nc.vector.reciprocal(ssum, ssum)
            a_tp = sml.tile([TP, G, R], F32, tag="atp")
            for g in range(G):
                nc.vector.tensor_scalar_mul(
                    a_tp[:, g, :], e_tp[:, g, :], ssum[:, g : g + 1]
                )

            # ---- alpha out ----
            nc.sync.dma_start(
                al_out.ap()[b].rearrange("(g t) r -> t g r", g=G), a_tp
            )

            # ---- alpha.T [49, 2, 128] and c_t matmuls ----
            aT = sml.tile([R, G, TP], BF16, tag="aT")
            for g in range(G):
                pat = ps_tr.tile([R, TP], F32, tag="tr")
                tpose(pat, a_tp[:, g, :], ident)
                evac(aT[:, g, :], pat)
            ctsb = outp.tile([TP, G, H], F32, tag="cts")
            for g in range(G):
                for nh in range(2):
                    pco = ps_co.tile([TP, 512], F32, tag="pco")
                    nc.tensor.matmul(
                        pco,
                        aT[:, g, :],
                        vnatb[:, nh * 512 : (nh + 1) * 512],
                        start=True,
                        stop=True,
                    )
                    evac(ctsb[:, g, nh * 512 : (nh + 1) * 512], pco)
            nc.sync.dma_start(
                ct_out.ap()[b].rearrange("(g t) h -> t g h", g=G), ctsb
            )

    _split_excess_waits(nc)
    return nc


# Instruction classes whose lowered form keeps more than one HW wait slot.
_WAIT_EXEMPT = ("InstEventSemaphore", "InstNoOp")


def _split_excess_waits(nc):
    """Each TPB instruction has a single HW (wait_mode, wait_idx, value) slot
    (a normal Matmult lowers to LDWEIGHTS+MATMUL = 2 slots).  Tile can emit
    more waits than that, which walrus rejects ("Too many sync wait
    commands").  Move the excess onto standalone InstEventSemaphore
    instructions placed immediately before, on the same engine."""
    import concourse.mybir as mybir

    uid = [0]
    f = nc.m.functions[0]
    for blk in f.blocks:
        il = blk.instructions
        out = []
        changed = False
        for inst in il:
            cls = inst.__class__.__name__
            si = inst.sync_info
            waits = list(si.on_wait) if si and si.on_wait else []
            if cls in _WAIT_EXEMPT:
                limit = 10**9
            else:
                limit = 1
            if len(waits) > limit:
                changed = True
                for w in waits[:-limit]:
                    uid[0] += 1
                    es = mybir.InstEventSemaphore(
                        name=f"esw-{uid[0]}",
                        engine=inst.engine,
                        ins=[],
                        outs=[],
                        sync_info=mybir.SyncInfo(on_wait=[w], on_update=[]),
                    )
                    out.append(es)
                inst.sync_info = mybir.SyncInfo(
                    on_wait=waits[-limit:],
                    on_update=list(si.on_update) if si.on_update else [],
                )
            out.append(inst)
        if changed:
            blk.instructions = out


def kernel(V, h_t, Wv, Wg, Wh):
    V = np.ascontiguousarray(V, dtype=np.float32)
    h_t = np.ascontiguousarray(h_t, dtype=np.float32)
    Wv = np.ascontiguousarray(Wv, dtype=np.float32)
    Wg = np.ascontiguousarray(Wg, dtype=np.float32)
    Wh = np.ascontiguousarray(Wh, dtype=np.float32)

    if "nc" not in _CACHE:
        _CACHE["nc"] = _build_program()
    nc = _CACHE["nc"]

    in_maps = []
    for c in range(NCORES):
        sl = slice(c * BPC, (c + 1) * BPC)
        in_maps.append(
            {"v": V[sl], "h": h_t[sl], "wv": Wv, "wg": Wg, "wh": Wh}
        )
    res = run_bass_kernel_spmd(
        nc,
        in_maps,
        core_ids=list(range(NCORES)),
        trace=bool(int(os.environ.get("KERNEL_TRACE", "0"))),
    )
    _CACHE["last_results"] = res
    ct = np.concatenate([r["ct"] for r in res.results], axis=0)
    al = np.concatenate([r["al"] for r in res.results], axis=0)
    return ct, al


if __name__ == "__main__":
    rng = np.random.default_rng(0)
    V = rng.standard_normal((B, R, H), dtype=np.float32)
    h_t = rng.standard_normal((B, T, H), dtype=np.float32)
    Wv = rng.standard_normal((R, H), dtype=np.float32) / np.sqrt(H)
    Wg = rng.standard_normal((R, H), dtype=np.float32) / np.sqrt(H)
    Wh = rng.standard_normal((1, R), dtype=np.float32) / np.sqrt(R)
    ct, al = kernel(V=V, h_t=h_t, Wv=Wv, Wg=Wg, Wh=Wh)
    print(ct.shape, al.shape, ct.dtype, al.dtype)


# revision 40
# speedup vs baseline: 1.1977x; 1.1977x over previous
"""Trainium2 Bass kernel for the visual-attention module.

reference:
    cv = einsum('brh,kh->brk', V, Wv)              # [B, R, R]
    cg = einsum('bth,kh->btk', h_t, Wg)            # [B, T, R]
    content = cv[:,None,:,:] + cg[:,:,None,:]      # [B, T, R, R]
    z = einsum('btrk,k->btr', tanh(content), Wh[0])
    alpha = softmax(z, -1)                         # [B, T, R]
    c_t = einsum('btr,brh->bth', alpha, V)         # [B, T, H]
    returns (c_t, alpha)

Sharding: data-parallel over batch B=64 across 8 cores (8 batches/core),
weights replicated.

Per-core dataflow (per batch b):
  - h_t/V loaded natural, PE-transposed to put H on partitions
  - cg.T computed "stacked": rows (g*64+k) for t-half g -> [113, 128] PSUM
  - content cube layout [113 partitions=(g,k), (r, t')] built by two
    broadcast-AP matmuls (identity-stack weights); tanh fused on ScalarE
    evacuating PSUM->SBUF
  - z-reduce: 13 matmuls with block-structured Wh weights accumulating into
    one [26, 512] PSUM bank (row pair 2j,2j+1 = r-chunk j)
  - z PE-transposed back to [t', (g, r)] layout, softmax there
  - alpha PE-transposed to [r, t] for the c_t matmuls against natural V
"""

import os
import sys

for _p in ("/opt/trn_rl_repo",):
    if _p not in sys.path:
        sys.path.insert(0, _p)

from contextlib import ExitStack

import numpy as np

import concourse.bass as bass
import concourse.tile as tile
from concourse import mybir
from concourse.bass_utils import run_bass_kernel_spmd

# problem sizes (hardcoded per spec)
B, T, R, H = 64, 256, 49, 1024
NCORES = 8
BPC = B // NCORES  # batches per core
G = 2  # t-halves (T = G*128)
TP = 128  # t' per half
KP = 64  # partition stride between the two g-blocks of k-rows
NP = KP + R  # 113 used partitions in the stacked layout
HC = H // 128  # h chunks
RC = 4  # r's per z-chunk (N = RC*TP = 512)
NZ = (R + RC - 1) // RC  # 13 z-chunks; last has 1 r
NDVE = 5  # trailing content groups computed on DVE instead of PE
F32 = mybir.dt.float32
BF16 = mybir.dt.bfloat16

_CACHE = {}


def _build_program():
    nc = bass.Bass()

    v_in = nc.dram_tensor("v", [BPC, R, H], F32, kind="ExternalInput")
    h_in = nc.dram_tensor("h", [BPC, T, H], F32, kind="ExternalInput")
    wv_in = nc.dram_tensor("wv", [R, H], F32, kind="ExternalInput")
    wg_in = nc.dram_tensor("wg", [R, H], F32, kind="ExternalInput")
    wh_in = nc.dram_tensor("wh", [1, R], F32, kind="ExternalInput")
    ct_out = nc.dram_tensor("ct", [BPC, T, H], F32, kind="ExternalOutput")
    al_out = nc.dram_tensor("al", [BPC, T, R], F32, kind="ExternalOutput")
    # DRAM staging for the bf16 copy of h_t; a single DRAM-source xbar
    # transpose per batch replaces 16 SBUF->SBUF 128x128 transposes
    hbf_d = nc.dram_tensor("hbf_scratch", [BPC, T, H], BF16)

    # inline constants
    ident_np = np.eye(128, dtype=np.float32)
    # lhsT_cg [113, 113]: identity on rows 0:49 and 64:113, zero elsewhere
    lcg_np = np.zeros((NP, NP), dtype=np.float32)
    for k in range(R):
        lcg_np[k, k] = 1.0
        lcg_np[KP + k, KP + k] = 1.0
    import ml_dtypes

    ident_d = nc.inline_tensor(ident_np, name="ident")
    lcg_d = nc.inline_tensor(lcg_np.astype(ml_dtypes.bfloat16), name="lcg")

    with tile.TileContext(nc) as tc, ExitStack() as ctx:
        const = ctx.enter_context(tc.tile_pool(name="const", bufs=1))
        wpool = ctx.enter_context(tc.tile_pool(name="wpool", bufs=1))
        inp = ctx.enter_context(tc.tile_pool(name="inp", bufs=3))
        trs = ctx.enter_context(tc.tile_pool(name="trs", bufs=3))
        cub = ctx.enter_context(tc.tile_pool(name="cub", bufs=3))
        sml = ctx.enter_context(tc.tile_pool(name="sml", bufs=3))
        outp = ctx.enter_context(tc.tile_pool(name="outp", bufs=2))
        ps_tr = ctx.enter_context(tc.tile_pool(name="ps_tr", bufs=2, space="PSUM"))
        ps_acc = ctx.enter_context(tc.tile_pool(name="ps_acc", bufs=2, space="PSUM"))
        ps_ct = ctx.enter_context(tc.tile_pool(name="ps_ct", bufs=2, space="PSUM"))
        ps_co = ctx.enter_context(tc.tile_pool(name="ps_co", bufs=2, space="PSUM"))

        ident = const.tile([128, 128], F32)
        nc.sync.dma_start(ident, ident_d.ap())
        lcg = const.tile([NP, NP], BF16)
        nc.sync.dma_start(lcg, lcg_d.ap())

        def tpose(out, in_, idap):
            return nc.tensor.transpose(out, in_, idap)

        def evac(dst, src):
            nc.vector.tensor_copy(dst, src)

        # --- transposed weights ---
        # wgtb [128, HC, 49] bf16 (cg matmuls run in bf16)
        # wvt2 [128, HC, 113] fp32 with Wv.T duplicated at cols 0:49 and
        # 64:113 so the cv matmul directly emits the stacked [113, 49] cv.T
        wgt = wpool.tile([128, HC, R], F32)
        wgtb = wpool.tile([128, HC, R], BF16)
        wvt2 = wpool.tile([128, HC, NP], F32)
        nc.vector.memset(wvt2, 0.0)
        for name, src, dsts in (("wg", wg_in, None), ("wv", wv_in, None)):
            wnat = inp.tile([R, H], F32, tag="wnat")
            nc.sync.dma_start(wnat, src.ap())
            for grp in range(2):
                pst = ps_tr.tile([128, 4, R], F32, tag="tr")
                for i in range(4):
                    hc = grp * 4 + i
                    tpose(
                        pst[:, i, :],
                        wnat[:, hc * 128 : (hc + 1) * 128],
                        ident[0:R, 0:R],
                    )
                sl = slice(grp * 4, (grp + 1) * 4)
                if name == "wg":
                    evac(wgt[:, sl, :], pst)
                else:
                    evac(wvt2[:, sl, 0:R], pst)
                    evac(wvt2[:, sl, KP:NP], pst)
        nc.vector.tensor_copy(wgtb, wgt)

        # --- Wh stacked column + z-reduce weights whT [113, NZ, 26] ---
        whs = wpool.tile([NP, 1], F32)
        nc.sync.dma_start(whs[0:R, :], wh_in.ap().rearrange("a b -> b a"))
        nc.sync.dma_start(whs[KP:NP, :], wh_in.ap().rearrange("a b -> b a"))
        wht = wpool.tile([NP, NZ, 2 * NZ], BF16)
        nc.vector.memset(wht, 0.0)
        for j in range(NZ):
            nc.vector.tensor_copy(wht[0:R, j, 2 * j : 2 * j + 1], whs[0:R, :])
            nc.vector.tensor_copy(wht[KP:NP, j, 2 * j + 1 : 2 * j + 2], whs[KP:NP, :])

        for b in range(BPC):
            # ---- load h_t natural, cast to bf16, DMA-xbar transpose to
            # htTb [128(h'), HC, 256(t)] bf16 ----
            htTb = trs.tile([128, HC, T], BF16, tag="htT")
            htn = inp.tile([128, G, H], F32, tag="htn")
            nc.sync.dma_start(
                htn, h_in.ap()[b].rearrange("(g t) h -> t g h", g=G)
            )
            hbf = inp.tile([128, G, H], BF16, tag="hbf")
            nc.gpsimd.tensor_copy(hbf, htn)
            nc.sync.dma_start(
                hbf_d.ap()[b].rearrange("(g t) h -> t g h", g=G), hbf
            )
            # one DRAM-source xbar transpose (kept alone on the ACT HWDGE
            # ring: mixing transpose/copy modes on a ring forces the
            # DMATranspose<->DMACopy serialization workaround):
            # out[p, hc, t] = h_t[t, hc*128+p]
            nc.scalar.dma_start_transpose(htTb, hbf_d.ap()[b])
            # ---- load V natural + transpose to vT [128, HC, 49] ----
            vnat = inp.tile([R, H], F32, tag="vnat")
            nc.sync.dma_start(vnat, v_in.ap()[b])
            vnatb = inp.tile([R, H], BF16, tag="vnatb")
            nc.gpsimd.tensor_copy(vnatb, vnat)
            vT = trs.tile([128, HC, R], F32, tag="vT")
            for grp in range(2):
                pst = ps_tr.tile([128, 4, R], F32, tag="tr")
                for i in range(4):
                    hc = grp * 4 + i
                    tpose(
                        pst[:, i, :],
                        vnat[:, hc * 128 : (hc + 1) * 128],
                        ident[0:R, 0:R],
                    )
                evac(vT[:, grp * 4 : (grp + 1) * 4, :], pst)

            # ---- cgT stacked [113, 128]: rows g*64+k = cg[b, g*128+t', k] ----
            pcg = ps_acc.tile([NP, TP], F32, tag="acc")
            for g in range(G):
                rows = pcg[0:R, :] if g == 0 else pcg[KP:NP, :]
                for hc in range(HC):
                    nc.tensor.matmul(
                        rows,
                        wgtb[:, hc, :],
                        htTb[:, hc, g * TP : (g + 1) * TP],
                        start=(hc == 0),
                        stop=(hc == HC - 1),
                    )
            # rows 49:64 of pcg are PSUM garbage; the DVE content chunks
            # read all 113 partitions, so zero the tile first and evacuate
            # the two valid row blocks (DVE partition base must be 32-aligned)
            cgs = sml.tile([NP, TP], BF16, tag="cgs")
            nc.gpsimd.memset(cgs, 0.0)
            evac(cgs[0:R, :], pcg[0:R, :])
            evac(cgs[KP:NP, :], pcg[KP:NP, :])

            # ---- cvT [49, 49] = cv[b].T ----
            pcv = ps_acc.tile([NP, R], F32, tag="acc")
            for hc in range(HC):
                nc.tensor.matmul(
                    pcv,
                    wvt2[:, hc, :],
                    vT[:, hc, 0:R],
                    start=(hc == 0),
                    stop=(hc == HC - 1),
                )
            cvt = sml.tile([NP, R], BF16, tag="cvt")
            evac(cvt, pcv)

            # ---- content cube + tanh: cube [113, 49, 128] ----
            cube = cub.tile([NP, R, TP], BF16, tag="cube")
            # PE: 4 groups of 4 r's (1 PSUM bank each); DVE: 2 groups of 16
            # plus the last r
            r_groups = [(0, 4, "pe"), (4, 4, "pe"), (8, 4, "pe"),
                        (12, 12, "dve"), (24, 12, "dve"), (36, 12, "dve"),
                        (48, 1, "dve")]
            for gi, (r0, rg, where) in enumerate(r_groups):
                if where == "pe":
                    pct = ps_ct.tile([NP, RC, TP], F32, tag="pct")
                    nsub = (rg + RC - 1) // RC
                    for i in range(nsub):
                        rr0 = i * RC
                        rc = min(RC, rg - rr0)
                        rhs_cv = (
                            cvt[:, r0 + rr0 : r0 + rr0 + rc]
                            .rearrange("k (r u) -> k r u", u=1)
                            .to_broadcast([NP, rc, TP])
                        )
                        nc.tensor.matmul(
                            pct[:, rr0 : rr0 + rc, :],
                            lcg,
                            rhs_cv,
                            start=True,
                            stop=False,
                        )
                    for i in range(nsub):
                        rr0 = i * RC
                        rc = min(RC, rg - rr0)
                        rhs_cg = cgs.rearrange(
                            "p (u t) -> p u t", u=1
                        ).to_broadcast([NP, rc, TP])
                        nc.tensor.matmul(
                            pct[:, rr0 : rr0 + rc, :],
                            lcg,
                            rhs_cg,
                            start=False,
                            stop=True,
                        )
                    nc.scalar.activation(
                        out=cube[:, r0 : r0 + rg, :],
                        in_=pct[:, 0:rg, :],
                        func=mybir.ActivationFunctionType.Tanh,
                    )
                else:
                    cdv = cub.tile([NP, 12, TP], BF16, tag="cdve")
                    in_cv = (
                        cvt[:, r0 : r0 + rg]
                        .rearrange("k (r u) -> k r u", u=1)
                        .to_broadcast([NP, rg, TP])
                    )
                    in_cg = cgs.rearrange("p (u t) -> p u t", u=1).to_broadcast(
                        [NP, rg, TP]
                    )
                    nc.vector.tensor_add(cdv[:, 0:rg, :], in_cv, in_cg)
                    nc.scalar.activation(
                        out=cube[:, r0 : r0 + rg, :],
                        in_=cdv[:, 0:rg, :],
                        func=mybir.ActivationFunctionType.Tanh,
                    )

            # ---- z-reduce: psz [26, 512], row 2j+g col (rr*128+t') ----
            psz = ps_co.tile([2 * NZ, RC * TP], F32, tag="pco")
            for j in range(NZ):
                rc = min(RC, R - j * RC)
                nc.tensor.matmul(
                    psz[:, 0 : rc * TP],
                    wht[:, j, :],
                    cube[:, j * RC : j * RC + rc, :],
                    start=(j == 0),
                    stop=(j == NZ - 1),
                )
            zs = sml.tile([2 * NZ, RC * TP], F32, tag="zs")
            evac(zs, psz)

            # ---- z back to t-partition layout: z_tp [128, 2, 52] ----
            z_tp = sml.tile([TP, G, 52], F32, tag="ztp")
            # 4 PE transposes into one PSUM tile, one strided evac; the
            # out-of-range (j=12, rr>0) lanes land in z_tp's pad columns 49:52
            pzt4 = ps_tr.tile([128, RC, 2 * NZ], F32, tag="tr")
            for rr in range(RC):
                tpose(
                    pzt4[:, rr, :],
                    zs[:, rr * TP : (rr + 1) * TP],
                    ident[0 : 2 * NZ, 0 : 2 * NZ],
                )
            z4 = z_tp.rearrange("p g (j u) -> p g j u", u=RC)
            evac(
                z4.rearrange("p g j u -> p u j g"),
                pzt4.rearrange("p u (j g) -> p u j g", g=G),
            )

            # ---- softmax over r (free dim), skip max-subtraction ----
            e_tp = sml.tile([TP, G, R], F32, tag="etp")
            nc.scalar.activation(
                out=e_tp,
                in_=z_tp[:, :, 0:R],
                func=mybir.ActivationFunctionType.Exp,
            )
            ssum = sml.tile([TP, G], F32, tag="ssum")
            nc.vector.tensor_reduce(
                out=ssum, in_=e_tp, axis=mybir.AxisListType.X, op=mybir.AluOpType.add
            )
            nc.vector.reciprocal(ssum, ssum)
            a_tp = sml.tile([TP, G, R], F32, tag="atp")
            for g in range(G):
                nc.vector.tensor_scalar_mul(
                    a_tp[:, g, :], e_tp[:, g, :], ssum[:, g : g + 1]
                )

            # ---- alpha out ----
            nc.sync.dma_start(
                al_out.ap()[b].rearrange("(g t) r -> t g r", g=G), a_tp
            )

            # ---- alpha.T [49, 2, 128] and c_t matmuls ----
            aT = sml.tile([R, G, TP], BF16, tag="aT")
            for g in range(G):
                pat = ps_tr.tile([R, TP], F32, tag="tr")
                tpose(pat, a_tp[:, g, :], ident)
                evac(aT[:, g, :], pat)
            ctsb = outp.tile([TP, G, H], F32, tag="cts")
            for g in range(G):
                for nh in range(2):
                    pco = ps_co.tile([TP, 512], F32, tag="pco")
                    nc.tensor.matmul(
                        pco,
                        aT[:, g, :],
                        vnatb[:, nh * 512 : (nh + 1) * 512],
                        start=True,
                        stop=True,
                    )
                    evac(ctsb[:, g, nh * 512 : (nh + 1) * 512], pco)
            nc.sync.dma_start(
                ct_out.ap()[b].rearrange("(g t) h -> t g h", g=G), ctsb
            )

    _split_excess_waits(nc)
    return nc


# Instruction classes whose lowered form keeps more than one HW wait slot.
_WAIT_EXEMPT = ("InstEventSemaphore", "InstNoOp")


def _split_excess_waits(nc):
    """Each TPB instruction has a single HW (wait_mode, wait_idx, value) slot
    (a normal Matmult lowers to LDWEIGHTS+MATMUL = 2 slots).  Tile can emit
    more waits than that, which walrus rejects ("Too many sync wait
    commands").  Move the excess onto standalone InstEventSemaphore
    instructions placed immediately before, on the same engine."""
    import concourse.mybir as mybir

    uid = [0]
    f = nc.m.functions[0]
    for blk in f.blocks:
        il = blk.instructions
        out = []
        changed = False
        for inst in il:
            cls = inst.__class__.__name__
            si = inst.sync_info
            waits = list(si.on_wait) if si and si.on_wait else []
            if cls in _WAIT_EXEMPT:
                limit = 10**9
            else:
                limit = 1
            if len(waits) > limit:
                changed = True
                for w in waits[:-limit]:
                    uid[0] += 1
                    es = mybir.InstEventSemaphore(
                        name=f"esw-{uid[0]}",
                        engine=inst.engine,
                        ins=[],
                        outs=[],
                        sync_info=mybir.SyncInfo(on_wait=[w], on_update=[]),
                    )
                    out.append(es)
                inst.sync_info = mybir.SyncInfo(
                    on_wait=waits[-limit:],
                    on_update=list(si.on_update) if si.on_update else [],
                )
            out.append(inst)
        if changed:
            blk.instructions = out


def kernel(V, h_t, Wv, Wg, Wh):
    V = np.ascontiguousarray(V, dtype=np.float32)
    h_t = np.ascontiguousarray(h_t, dtype=np.float32)
    Wv = np.ascontiguousarray(Wv, dtype=np.float32)
    Wg = np.ascontiguousarray(Wg, dtype=np.float32)
    Wh = np.ascontiguousarray(Wh, dtype=np.float32)

    if "nc" not in _CACHE:
        _CACHE["nc"] = _build_program()
    nc = _CACHE["nc"]

    in_maps = []
    for c in range(NCORES):
        sl = slice(c * BPC, (c + 1) * BPC)
        in_maps.append(
            {"v": V[sl], "h": h_t[sl], "wv": Wv, "wg": Wg, "wh": Wh}
        )
    res = run_bass_kernel_spmd(
        nc,
        in_maps,
        core_ids=list(range(NCORES)),
        trace=bool(int(os.environ.get("KERNEL_TRACE", "0"))),
    )
    _CACHE["last_results"] = res
    ct = np.concatenate([r["ct"] for r in res.results], axis=0)
    al = np.concatenate([r["al"] for r in res.results], axis=0)
    return ct, al


if __name__ == "__main__":
    rng = np.random.default_rng(0)
    V = rng.standard_normal((B, R, H), dtype=np.float32)
    h_t = rng.standard_normal((B, T, H), dtype=np.float32)
    Wv = rng.standard_normal((R, H), dtype=np.float32) / np.sqrt(H)
    Wg = rng.standard_normal((R, H), dtype=np.float32) / np.sqrt(H)
    Wh = rng.standard_normal((1, R), dtype=np.float32) / np.sqrt(R)
    ct, al = kernel(V=V, h_t=h_t, Wv=Wv, Wg=Wg, Wh=Wh)
    print(ct.shape, al.shape, ct.dtype, al.dtype)
